# revision 1
# baseline (speedup 1.0000x reference)
"""Trainium2 Bass kernel for fused attention (QKV proj + RoPE + SDPA + o_proj).

Sharding: Megatron-style tensor parallel over heads (4 heads/core x 8 cores)
for QKV+SDPA, then an AllToAll switches to token parallelism for o_proj, so
each core emits a disjoint slice of the final output (host just concatenates).

All device matmuls run as float32r (full-rate fp32 on the PE array, ~1e-4 rel).
Activations stay in transposed [e, t] layouts end-to-end so no on-device
activation transposes are needed.
"""
import sys

import numpy as np

try:
    import concourse.bass as bass
except ImportError:  # fresh grading env: make the toolchain importable
    for p in (
        "/root/.axon_site",
        "/root/.axon_site/_ro/trn_rl_repo",
        "/root/.axon_site/_ro/pypackages",
        "/opt/trn_rl_repo",
        "/opt/pypackages",
    ):
        if p not in sys.path:
            sys.path.append(p)
    import concourse.bass as bass

import concourse.bacc as bacc
import concourse.mybir as mybir
import concourse.tile as tile
from concourse.bass_utils import run_bass_kernel_spmd

F32 = mybir.dt.float32
F32R = mybir.dt.float32r
MULT = mybir.AluOpType.mult
ADD = mybir.AluOpType.add

# problem dims (hardcoded for nn_Attention_42846593744909)
B, S, D = 4, 1024, 2048
H, HD = 32, 64
N_CORES = 8
H_LOC = H // N_CORES  # heads per core


def build_attention(b=B, s=S, d=D, h_loc=H_LOC, hd=HD, n_cores=N_CORES):
    """Build the per-core SPMD Bass program. Returns finalized nc."""
    P = 128
    T = b * s                 # total tokens
    TS = T // n_cores         # output token slice per core
    DCH = d // P              # contraction chunks for D
    QBLK = h_loc * hd         # 256: q (or k, or v) width per core
    NQKQ = QBLK // P          # q e-chunks (2)
    NQK = 2 * NQKQ            # q+k e-chunks (4)
    EVA = h_loc * (hd + 1)    # v + ones columns (260)
    TCH = min(256, s)         # qkv token chunk
    NTC = s // TCH
    QT = min(512, s, TS)      # query-tile width in SDPA
    NQT = s // QT
    KTC = s // P              # key chunks of 128
    ECH = n_cores * QBLK // P  # o_proj contraction chunks (16)
    NH = 2                    # number of A2A rounds (token halves)
    SH = T // (NH * n_cores)  # shard tokens per core per half
    TS_H = TS // NH
    ODC = min(256, d)         # o_proj dout chunk
    TSUB = TS // P            # o_proj token subchunks
    NP = min(NH, TSUB)        # o_proj passes
    assert QT <= TS and TS % QT == 0 and QT % SH == 0 and SH == TS_H
    assert TSUB % NP == 0

    nc = bacc.Bacc()
    hidden_t = nc.dram_tensor("hidden_t", [d, T], F32R, kind="ExternalInput")
    w_qk_t = nc.dram_tensor("w_qk_t", [d, 2 * QBLK], F32R, kind="ExternalInput")
    w_v_t = nc.dram_tensor("w_v_t", [d, QBLK], F32R, kind="ExternalInput")
    w_o_t = nc.dram_tensor("w_o_t", [n_cores * QBLK, d], F32R, kind="ExternalInput")
    cos2 = nc.dram_tensor("cos2", [P, s], F32, kind="ExternalInput")
    sinrot2 = nc.dram_tensor("sinrot2", [P, s], F32, kind="ExternalInput")
    out_sl = nc.dram_tensor("out_sl", [TS, d], F32, kind="ExternalOutput")

    hid_v = hidden_t[:].rearrange("(c p) t -> p c t", p=P)
    wqk_v = w_qk_t[:].rearrange("(c p) e -> p c e", p=P)
    wv_v = w_v_t[:].rearrange("(c p) e -> p c e", p=P)
    wo_v = w_o_t[:].rearrange("(c p) e -> p c e", p=P)

    with tile.TileContext(nc) as tc:
        with tc.tile_pool(name="dramp", bufs=1, space="DRAM") as dramp:
            cc_in_h = [dramp.tile([n_cores, QBLK, SH], F32, name=f"cc_in_{h}")
                       for h in range(NH)]
            cc_out_h = [dramp.tile([n_cores, QBLK, SH], F32, name=f"cc_out_{h}")
                        for h in range(NH)]

            with (
                tc.tile_pool(name="tabs", bufs=1) as tabs,
                tc.tile_pool(name="hidp", bufs=2) as hidp,
                tc.tile_pool(name="qkp", bufs=2) as qkp,
                tc.tile_pool(name="vp", bufs=2) as vp,
                tc.tile_pool(name="ropep", bufs=2) as ropep,
                tc.tile_pool(name="expp", bufs=3) as expp,
                tc.tile_pool(name="normp", bufs=2) as normp,
                tc.tile_pool(name="drowp", bufs=4, space="DRAM") as drowp,
                tc.tile_pool(name="psA", bufs=2, space="PSUM") as psA,
                tc.tile_pool(name="psS", bufs=3, space="PSUM") as psS,
                tc.tile_pool(name="psO", bufs=3, space="PSUM") as psO,
            ):
                cos_sb = tabs.tile([P, s], F32)
                sin_sb = tabs.tile([P, s], F32)
                nc.sync.dma_start(cos_sb[:], cos2[:])
                nc.sync.dma_start(sin_sb[:], sinrot2[:])

                with tc.tile_pool(name="wqkp", bufs=1) as wqkp:
                    wqk_sb = wqkp.tile([P, DCH, 2 * QBLK], F32R)
                    wv_sb = wqkp.tile([P, DCH, QBLK], F32R)
                    wstep = max(1, DCH // 4)
                    for dd4 in range(0, DCH, wstep):
                        nc.sync.dma_start(wqk_sb[:, dd4:dd4 + wstep],
                                          wqk_v[:, dd4:dd4 + wstep])
                        nc.sync.dma_start(wv_sb[:, dd4:dd4 + wstep],
                                          wv_v[:, dd4:dd4 + wstep])

                    for bi in range(b):
                        # ---- QKV projection + RoPE for batch bi ----
                        qk_t = qkp.tile([P, NQK, s], F32R, tag="qk")
                        v_t = vp.tile([P, KTC, EVA], F32R, tag="v")
                        for h in range(h_loc):
                            nc.scalar.activation(
                                v_t[:, :, h * (hd + 1) + hd:h * (hd + 1) + hd + 1],
                                wv_sb[:, 0:KTC, 0:1],
                                mybir.ActivationFunctionType.Identity,
                                bias=1.0, scale=0.0,
                            )

                        for tci in range(NTC):
                            t0 = bi * s + tci * TCH
                            s0 = tci * TCH
                            hid_sb = hidp.tile([P, DCH, TCH], F32R, tag="hid")
                            nc.sync.dma_start(hid_sb[:], hid_v[:, :, t0:t0 + TCH])

                            for ec in range(NQK):
                                ps = psA.tile([P, max(QBLK, TCH)], F32, tag="psqk",
                                              name="psqk")[:, 0:TCH]
                                for dd in range(DCH):
                                    nc.tensor.matmul(
                                        ps[:], lhsT=wqk_sb[:, dd, ec * P:(ec + 1) * P],
                                        rhs=hid_sb[:, dd, :],
                                        start=(dd == 0), stop=(dd == DCH - 1),
                                    )
                                # RoPE: out = ps*cos + swap32(ps)*sinrot
                                raw = ropep.tile([P, TCH], F32, tag="raw")
                                nc.any.tensor_copy(raw[:], ps[:])
                                cp = ropep.tile([P, TCH], F32, tag="cp")
                                nc.vector.tensor_tensor(cp[:], ps[:], cos_sb[:, s0:s0 + TCH], MULT)
                                sw = ropep.tile([P, TCH], F32, tag="sw")
                                nc.sync.dma_start(sw[0:32, :], raw[32:64, :])
                                nc.sync.dma_start(sw[32:64, :], raw[0:32, :])
                                nc.sync.dma_start(sw[64:96, :], raw[96:128, :])
                                nc.sync.dma_start(sw[96:128, :], raw[64:96, :])
                                nc.vector.tensor_tensor(sw[:], sw[:], sin_sb[:, s0:s0 + TCH], MULT)
                                nc.vector.tensor_tensor(qk_t[:, ec, s0:s0 + TCH], cp[:], sw[:], ADD)

                            for tsub in range(TCH // P):
                                kc = tci * (TCH // P) + tsub
                                psv = psA.tile([P, max(QBLK, TCH)], F32, tag="psqk",
                                               name="psv")[:, 0:QBLK]
                                for dd in range(DCH):
                                    nc.tensor.matmul(
                                        psv[:], lhsT=hid_sb[:, dd, tsub * P:(tsub + 1) * P],
                                        rhs=wv_sb[:, dd, :],
                                        start=(dd == 0), stop=(dd == DCH - 1),
                                    )
                                for h in range(h_loc):
                                    nc.any.tensor_copy(
                                        v_t[:, kc, h * (hd + 1):h * (hd + 1) + hd],
                                        psv[:, h * hd:(h + 1) * hd],
                                    )

                        # ---- SDPA for batch bi ----
                        for pp in range(h_loc // 2):
                            for qt in range(NQT):
                                q0 = qt * QT
                                ps_o0 = psO.tile([P, QT], F32, tag="pso")
                                ps_o1 = psO.tile([P, QT], F32, tag="pso")
                                for kt in range(KTC):
                                    ps_s0 = psS.tile([P, QT], F32, tag="pss")
                                    ps_s1 = psS.tile([P, QT], F32, tag="pss")
                                    nc.tensor.matmul(
                                        ps_s0[:],
                                        lhsT=qk_t[0:64, NQKQ + pp, kt * P:(kt + 1) * P],
                                        rhs=qk_t[0:64, pp, q0:q0 + QT],
                                        start=True, stop=True,
                                    )
                                    nc.tensor.matmul(
                                        ps_s1[:],
                                        lhsT=qk_t[64:128, NQKQ + pp, kt * P:(kt + 1) * P],
                                        rhs=qk_t[64:128, pp, q0:q0 + QT],
                                        start=True, stop=True, tile_position=(64, 0),
                                    )
                                    e0 = expp.tile([P, QT], F32R, tag="exp")
                                    e1 = expp.tile([P, QT], F32R, tag="exp")
                                    nc.scalar.activation(e0[:], ps_s0[:], mybir.ActivationFunctionType.Exp)
                                    nc.scalar.activation(e1[:], ps_s1[:], mybir.ActivationFunctionType.Exp)
                                    h0 = 2 * pp
                                    h1 = 2 * pp + 1
                                    nc.tensor.matmul(
                                        ps_o0[0:hd + 1, :],
                                        lhsT=v_t[:, kt, h0 * (hd + 1):(h0 + 1) * (hd + 1)],
                                        rhs=e0[:],
                                        start=(kt == 0), stop=(kt == KTC - 1),
                                    )
                                    nc.tensor.matmul(
                                        ps_o1[0:hd + 1, :],
                                        lhsT=v_t[:, kt, h1 * (hd + 1):(h1 + 1) * (hd + 1)],
                                        rhs=e1[:],
                                        start=(kt == 0), stop=(kt == KTC - 1),
                                    )
                                gq0 = bi * s + qt * QT
                                # pair-batched softmax denominators
                                dcp0 = normp.tile([hd + 1, QT], F32, tag="dcp")
                                dcp1 = normp.tile([hd + 1, QT], F32, tag="dcp")
                                nc.scalar.copy(dcp0[hd:hd + 1, :], ps_o0[hd:hd + 1, :])
                                nc.scalar.copy(dcp1[hd:hd + 1, :], ps_o1[hd:hd + 1, :])
                                dg = normp.tile([2, QT], F32, tag="dg")
                                nc.sync.dma_start(dg[0:1, :], dcp0[hd:hd + 1, :])
                                nc.sync.dma_start(dg[1:2, :], dcp1[hd:hd + 1, :])
                                dgr = normp.tile([2, QT], F32, tag="dgr")
                                nc.vector.reciprocal(dgr[:], dg[:])
                                rd = drowp.tile([2, QT], F32, tag="drow")
                                nc.sync.dma_start(rd[:], dgr[:])
                                for idx, (hh, ps_o) in enumerate(
                                        ((2 * pp, ps_o0), (2 * pp + 1, ps_o1))):
                                    rep = normp.tile([hd, QT], F32, tag="rep")
                                    nc.sync.dma_start(rep[:], rd[idx:idx + 1, :].to_broadcast((hd, QT)))
                                    ao = normp.tile([hd, QT], F32, tag="ao")
                                    nc.vector.tensor_tensor(ao[:], ps_o[0:hd, :], rep[:], MULT)
                                    for w in range(QT // SH):
                                        tok0 = gq0 + w * SH
                                        half = tok0 // (T // NH)
                                        o = tok0 % (T // NH)
                                        nc.sync.dma_start(
                                            cc_in_h[half][o // SH, hh * hd:(hh + 1) * hd, :],
                                            ao[:, w * SH:(w + 1) * SH],
                                        )

                # wqk/wv SBUF released here -> o_proj pools can alias it
                for h in range(NH):
                    nc.gpsimd.collective_compute(
                        "AllToAll",
                        mybir.AluOpType.bypass,
                        replica_groups=[list(range(n_cores))],
                        ins=[cc_in_h[h].opt()],
                        outs=[cc_out_h[h].opt()],
                    )

                # ---- o_proj on this core's token slice, NP passes ----
                with (
                    tc.tile_pool(name="aslp", bufs=1) as aslp,
                    tc.tile_pool(name="wop", bufs=2) as wop,
                    tc.tile_pool(name="osbp", bufs=3) as osbp,
                ):
                    asl = aslp.tile([P, ECH, TS], F32R)
                    for h in range(NH):
                        cc_v = cc_out_h[h][:].rearrange("j (ci p) t -> p (j ci) t", p=P)
                        nc.sync.dma_start(asl[:, :, h * TS_H:(h + 1) * TS_H], cc_v.bitcast(F32R))
                    for hp in range(NP):
                        for dc in range(d // ODC):
                            wo_sb = wop.tile([P, ECH, ODC], F32R, tag="wo")
                            nc.sync.dma_start(wo_sb[:], wo_v[:, :, dc * ODC:(dc + 1) * ODC])
                            for tsub in range(hp * (TSUB // NP), (hp + 1) * (TSUB // NP)):
                                pso = psA.tile([P, max(QBLK, TCH)], F32, tag="psqk",
                                               name="pso")[:, 0:ODC]
                                for e in range(ECH):
                                    nc.tensor.matmul(
                                        pso[:], lhsT=asl[:, e, tsub * P:(tsub + 1) * P],
                                        rhs=wo_sb[:, e, :],
                                        start=(e == 0), stop=(e == ECH - 1),
                                    )
                                ob = osbp.tile([P, ODC], F32, tag="ob")
                                nc.scalar.copy(ob[:], pso[:])
                                nc.sync.dma_start(
                                    out_sl[tsub * P:(tsub + 1) * P, dc * ODC:(dc + 1) * ODC], ob[:]
                                )
    nc.finalize()
    return nc



def prep_inputs(cos, sin, hidden_states, w_qkv, w_o,
                b=B, s=S, d=D, h_loc=H_LOC, hd=HD, n_cores=N_CORES):
    """Host-side sharding/layout: returns per-core input maps."""
    cos = np.asarray(cos, dtype=np.float32)
    sin = np.asarray(sin, dtype=np.float32)
    hidden_states = np.asarray(hidden_states, dtype=np.float32)
    w_qkv = np.asarray(w_qkv, dtype=np.float32)
    w_o = np.asarray(w_o, dtype=np.float32)

    T = b * s
    QBLK = h_loc * hd
    HHD = n_cores * QBLK  # total H*HD

    hidden_t = np.ascontiguousarray(hidden_states.reshape(T, d).T)
    w_o_t = np.ascontiguousarray(w_o.T)

    cos_t = cos.T  # [hd, s]
    sin_t = sin.T
    cos2 = np.ascontiguousarray(np.tile(cos_t, (128 // hd, 1)))
    srt = sin_t.copy()
    srt[0:hd // 2] = -sin_t[0:hd // 2]
    sinrot2 = np.ascontiguousarray(np.tile(srt, (128 // hd, 1)))

    maps = []
    for c in range(n_cores):
        wq = w_qkv[c * QBLK:(c + 1) * QBLK] * 0.125
        wk = w_qkv[HHD + c * QBLK:HHD + (c + 1) * QBLK]
        wv = w_qkv[2 * HHD + c * QBLK:2 * HHD + (c + 1) * QBLK]
        w_qk_t = np.ascontiguousarray(np.concatenate([wq, wk], axis=0).T)
        w_v_t = np.ascontiguousarray(wv.T)
        maps.append({
            "hidden_t": hidden_t,
            "w_qk_t": w_qk_t,
            "w_v_t": w_v_t,
            "w_o_t": w_o_t,
            "cos2": cos2,
            "sinrot2": sinrot2,
        })
    return maps


_NC_CACHE = {}


def run(inputs, trace=False, dims=None):
    """Run the distributed kernel. Returns (full_output, BassKernelResults)."""
    dims = dims or dict(b=B, s=S, d=D, h_loc=H_LOC, hd=HD, n_cores=N_CORES)
    key = tuple(sorted(dims.items()))
    if key not in _NC_CACHE:
        _NC_CACHE[key] = build_attention(**dims)
    nc = _NC_CACHE[key]
    maps = prep_inputs(inputs["cos"], inputs["sin"], inputs["hidden_states"],
                       inputs["w_qkv"], inputs["w_o"], **dims)
    res = run_bass_kernel_spmd(nc, maps, list(range(dims["n_cores"])), trace=trace)
    n_cores = dims["n_cores"]
    T = dims["b"] * dims["s"]
    TS_H = T // (2 * n_cores)
    out = np.empty((T, dims["d"]), dtype=np.float32)
    for c in range(n_cores):
        sl = res.results[c]["out_sl"]
        for h in range(2):
            out[h * (T // 2) + c * TS_H: h * (T // 2) + (c + 1) * TS_H] =                 sl[h * TS_H:(h + 1) * TS_H]
    out = out.reshape(dims["b"], dims["s"], dims["d"])
    return out, res


def kernel(**inputs) -> np.ndarray:
    out, _ = run(inputs)
    return out



# revision 7
# speedup vs baseline: 1.2105x; 1.2105x over previous
"""Trainium2 Bass kernel for fused attention (QKV proj + RoPE + SDPA + o_proj).

Sharding: Megatron-style tensor parallel over heads (4 heads/core x 8 cores)
for QKV+SDPA, then per-batch AllToAll rounds switch to token parallelism for
o_proj, so each core emits a disjoint slice of the final output.

v2 design vs baseline:
  - all matmuls in bf16 (halves DMA + SBUF; psum accumulate stays f32)
  - w_o resident in SBUF (bf16, 64KB/part) instead of streamed twice as f32
  - ap=512 moving rows everywhere; RoPE batched per [128,512] tile
  - AllToAll split into 4 per-batch rounds, each overlapped with the next
    batch's compute; o_proj matmuls interleaved into the (Act-limited) SDPA
    loop so the tensor stream stays dense; tail is only the last A2A+o_proj
  - softmax denominators via ones-column in V; reciprocal_approx_fast
"""
import sys

import numpy as np

try:
    import concourse.bass as bass
except ImportError:  # fresh grading env: make the toolchain importable
    for p in (
        "/root/.axon_site",
        "/root/.axon_site/_ro/trn_rl_repo",
        "/root/.axon_site/_ro/pypackages",
        "/opt/trn_rl_repo",
        "/opt/pypackages",
    ):
        if p not in sys.path:
            sys.path.append(p)
    import concourse.bass as bass

import concourse.bacc as bacc
import concourse.mybir as mybir
import concourse.tile as tile
from concourse.bass_utils import run_bass_kernel_spmd

import ml_dtypes

F32 = mybir.dt.float32
BF16 = mybir.dt.bfloat16
MULT = mybir.AluOpType.mult
ADD = mybir.AluOpType.add
EXP = mybir.ActivationFunctionType.Exp
IDENT = mybir.ActivationFunctionType.Identity

# problem dims (hardcoded for nn_Attention_42846593744909)
B, S, D = 4, 1024, 2048
H, HD = 32, 64
N_CORES = 8
H_LOC = H // N_CORES  # heads per core


def build_attention(b=B, s=S, d=D, h_loc=H_LOC, hd=HD, n_cores=N_CORES):
    """Build the per-core SPMD Bass program. Returns finalized nc."""
    P = 128
    T = b * s                  # total tokens
    DCH = d // P               # contraction chunks for D (16)
    QBLK = h_loc * hd          # 256: q (or k, or v) width per core
    NQK = 2 * QBLK // P        # q+k e-chunks (4)
    EVA = h_loc * (hd + 1)     # v + ones columns (260)
    TH = 512                   # proj token half-batch
    NTH = s // TH              # halves per batch (2)
    QT = 512                   # query tile in SDPA
    NQT = s // QT              # 2
    KTC = s // P               # key chunks of 128 (8)
    ECH = n_cores * QBLK // P  # o_proj contraction chunks (16)
    RT = s // n_cores          # round tokens per core per batch (128)
    ODC = 512                  # o_proj dout chunk (psum bank)
    NDC = d // ODC             # 4
    TS = b * RT                # total output tokens per core (512)

    nc = bacc.Bacc()
    hidden_t = nc.dram_tensor("hidden_t", [d, T], BF16, kind="ExternalInput")
    w_qk_t = nc.dram_tensor("w_qk_t", [d, 2 * QBLK], BF16, kind="ExternalInput")
    w_v_t = nc.dram_tensor("w_v_t", [d, QBLK], BF16, kind="ExternalInput")
    w_o_t = nc.dram_tensor("w_o_t", [n_cores * QBLK, d], BF16, kind="ExternalInput")
    cos2 = nc.dram_tensor("cos2", [P, s], F32, kind="ExternalInput")
    sinrot2 = nc.dram_tensor("sinrot2", [P, s], F32, kind="ExternalInput")
    out_sl = nc.dram_tensor("out_sl", [TS, d], F32, kind="ExternalOutput")

    hid_v = hidden_t[:].rearrange("(c p) t -> p c t", p=P)
    wqk_v = w_qk_t[:].rearrange("(c p) e -> p c e", p=P)
    wv_v = w_v_t[:].rearrange("(c p) e -> p c e", p=P)
    wo_v = w_o_t[:].rearrange("(c p) e -> p c e", p=P)

    with tile.TileContext(nc) as tc:
        with tc.tile_pool(name="dramp", bufs=1, space="DRAM") as dramp:
            cc_in = [dramp.tile([n_cores, QBLK, RT], BF16, name=f"cc_in_{r}")
                     for r in range(b)]
            cc_out = [dramp.tile([n_cores, QBLK, RT], BF16, name=f"cc_out_{r}")
                      for r in range(b)]
            # scatter view: [dst core j, pp, v, h, t]
            ccin_v = [t_[:].rearrange("j (pp h v) t -> j pp v h t", pp=2, h=2, v=hd)
                      for t_ in cc_in]
            ccout_v = [t_[:].rearrange("j (ci p) t -> p (j ci) t", p=P)
                       for t_ in cc_out]

            with (
                tc.tile_pool(name="tabs", bufs=1) as tabs,
                tc.tile_pool(name="hidp", bufs=2) as hidp,
                tc.tile_pool(name="qkp", bufs=2) as qkp,
                tc.tile_pool(name="vp", bufs=2) as vp,
                tc.tile_pool(name="ropep", bufs=2) as ropep,
                tc.tile_pool(name="expp", bufs=3) as expp,
                tc.tile_pool(name="aop", bufs=2) as aop,
                tc.tile_pool(name="dgp", bufs=1) as dgp,
                tc.tile_pool(name="repp", bufs=2) as repp,
                tc.tile_pool(name="aonp", bufs=2) as aonp,
                tc.tile_pool(name="aslp", bufs=2) as aslp,
                tc.tile_pool(name="obp", bufs=1) as obp,
                tc.tile_pool(name="drowp", bufs=4, space="DRAM") as drowp,
                tc.tile_pool(name="psP", bufs=2, space="PSUM") as psP,
                tc.tile_pool(name="psS", bufs=2, space="PSUM") as psS,
                tc.tile_pool(name="psO", bufs=2, space="PSUM") as psO,
                tc.tile_pool(name="psJ", bufs=2, space="PSUM") as psJ,
            ):
                # ---- static tables / weights (issue order = priority) ----
                wqk_sb = tabs.tile([P, DCH, 2 * QBLK], BF16)
                # first-needed chunk first: ec0 columns
                for ec in range(NQK):
                    nc.sync.dma_start(wqk_sb[:, :, ec * P:(ec + 1) * P],
                                      wqk_v[:, :, ec * P:(ec + 1) * P])

                def load_hid(bi):
                    tiles = []
                    for th in range(NTH):
                        t0 = bi * s + th * TH
                        hid_sb = hidp.tile([P, DCH, TH], BF16, tag="hid", name="hid")
                        nc.sync.dma_start(hid_sb[:], hid_v[:, :, t0:t0 + TH])
                        tiles.append(hid_sb)
                    return tiles

                hid_next = load_hid(0)
                cos_sb = tabs.tile([P, s], F32)
                sin_sb = tabs.tile([P, s], F32)
                nc.sync.dma_start(cos_sb[:], cos2[:])
                nc.sync.dma_start(sin_sb[:], sinrot2[:])
                wv_sb = tabs.tile([P, DCH, QBLK], BF16)
                nc.sync.dma_start(wv_sb[:], wv_v[:])
                wo_sb = tabs.tile([P, ECH, d], BF16)
                for dc in range(8):
                    nc.sync.dma_start(wo_sb[:, :, dc * 256:(dc + 1) * 256],
                                      wo_v[:, :, dc * 256:(dc + 1) * 256])

                def rope(ps, soff, qk_t, ec):
                    """RoPE a [128, TH] psum tile into qk_t[:, ec, soff:soff+TH]."""
                    raw = ropep.tile([P, TH], F32, tag="raw", name="raw")
                    nc.scalar.copy(raw[:], ps[:])
                    cp = ropep.tile([P, TH], F32, tag="cp", name="cp")
                    nc.vector.tensor_tensor(cp[:], ps[:], cos_sb[:, soff:soff + TH], MULT)
                    sw = ropep.tile([P, TH], F32, tag="sw", name="sw")
                    # rotate_half: swap 32-partition blocks within each head
                    nc.scalar.dma_start(sw[0:32, :], raw[32:64, :])
                    nc.scalar.dma_start(sw[32:64, :], raw[0:32, :])
                    nc.scalar.dma_start(sw[64:96, :], raw[96:128, :])
                    nc.scalar.dma_start(sw[96:128, :], raw[64:96, :])
                    nc.vector.tensor_tensor(sw[:], sw[:], sin_sb[:, soff:soff + TH], MULT)
                    nc.vector.tensor_tensor(qk_t[:, ec, soff:soff + TH], cp[:], sw[:], ADD)

                def proj(bi, qk_t, v_t, hid_tiles):
                    """QKV projection + RoPE for batch bi."""
                    # ones columns for the softmax denominator
                    for h in range(h_loc):
                        nc.scalar.activation(
                            v_t[:, :, h * (hd + 1) + hd:h * (hd + 1) + hd + 1],
                            wv_sb[:, 0:KTC, 0:1], IDENT, bias=1.0, scale=0.0)
                    for th in range(NTH):
                        s0 = th * TH
                        hid_sb = hid_tiles[th]
                        for ec in range(NQK):
                            ps = psP.tile([P, ODC], F32, tag="psP", name="psqk")
                            for dd in range(DCH):
                                nc.tensor.matmul(
                                    ps[:], lhsT=wqk_sb[:, dd, ec * P:(ec + 1) * P],
                                    rhs=hid_sb[:, dd, :],
                                    start=(dd == 0), stop=(dd == DCH - 1))
                            rope(ps, s0, qk_t, ec)
                        for tsub in range(TH // P):
                            kc = th * (TH // P) + tsub
                            psv = psP.tile([P, ODC], F32, tag="psP", name="psv")
                            for dd in range(DCH):
                                nc.tensor.matmul(
                                    psv[:, 0:QBLK],
                                    lhsT=hid_sb[:, dd, tsub * P:(tsub + 1) * P],
                                    rhs=wv_sb[:, dd, :],
                                    start=(dd == 0), stop=(dd == DCH - 1))
                            for h in range(h_loc):
                                nc.scalar.copy(
                                    v_t[:, kc, h * (hd + 1):h * (hd + 1) + hd],
                                    psv[:, h * hd:(h + 1) * hd])

                def oproj_gen(r):
                    """Generator emitting o_proj for round r in small steps."""
                    asl = aslp.tile([P, ECH, RT], BF16, tag="asl", name=f"asl{r}")
                    nc.sync.dma_start(asl[:], ccout_v[r])
                    yield
                    for dcg in range(NDC // 2):
                        pj0 = psJ.tile([P, ODC], F32, tag="psJ", name="pj0")
                        pj1 = psJ.tile([P, ODC], F32, tag="psJ", name="pj1")
                        d0 = (2 * dcg) * ODC
                        d1 = (2 * dcg + 1) * ODC
                        for e in range(ECH):
                            nc.tensor.matmul(pj0[:], lhsT=asl[:, e, :],
                                             rhs=wo_sb[:, e, d0:d0 + ODC],
                                             start=(e == 0), stop=(e == ECH - 1))
                            nc.tensor.matmul(pj1[:], lhsT=asl[:, e, :],
                                             rhs=wo_sb[:, e, d1:d1 + ODC],
                                             start=(e == 0), stop=(e == ECH - 1))
                            yield
                        ob = obp.tile([P, 2 * ODC], F32, tag="ob", name="ob")
                        nc.scalar.copy(ob[:, 0:ODC], pj0[:])
                        nc.scalar.copy(ob[:, ODC:2 * ODC], pj1[:])
                        nc.sync.dma_start(
                            out_sl[r * RT:(r + 1) * RT, d0:d0 + 2 * ODC], ob[:])
                        yield

                def sdpa(bi, qk_t, v_t, feeder):
                    """SDPA for batch bi with o_proj steps interleaved."""
                    for pp in range(h_loc // 2):
                        for qt in range(NQT):
                            q0 = qt * QT
                            ps_o0 = psO.tile([P, QT], F32, tag="psO", name="pso0")
                            ps_o1 = psO.tile([P, QT], F32, tag="psO", name="pso1")
                            for kt in range(KTC):
                                ps_s0 = psS.tile([P, QT], F32, tag="psS", name="pss0")
                                ps_s1 = psS.tile([P, QT], F32, tag="psS", name="pss1")
                                nc.tensor.matmul(
                                    ps_s0[:],
                                    lhsT=qk_t[0:64, 2 + pp, kt * P:(kt + 1) * P],
                                    rhs=qk_t[0:64, pp, q0:q0 + QT],
                                    start=True, stop=True)
                                nc.tensor.matmul(
                                    ps_s1[:],
                                    lhsT=qk_t[64:128, 2 + pp, kt * P:(kt + 1) * P],
                                    rhs=qk_t[64:128, pp, q0:q0 + QT],
                                    start=True, stop=True, tile_position=(64, 0))
                                e0 = expp.tile([P, QT], BF16, tag="exp", name="e0")
                                e1 = expp.tile([P, QT], BF16, tag="exp", name="e1")
                                nc.scalar.activation(e0[:], ps_s0[:], EXP)
                                nc.scalar.activation(e1[:], ps_s1[:], EXP)
                                h0 = 2 * pp
                                h1 = 2 * pp + 1
                                nc.tensor.matmul(
                                    ps_o0[0:hd + 1, :],
                                    lhsT=v_t[:, kt, h0 * (hd + 1):(h0 + 1) * (hd + 1)],
                                    rhs=e0[:],
                                    start=(kt == 0), stop=(kt == KTC - 1))
                                nc.tensor.matmul(
                                    ps_o1[0:hd + 1, :],
                                    lhsT=v_t[:, kt, h1 * (hd + 1):(h1 + 1) * (hd + 1)],
                                    rhs=e1[:],
                                    start=(kt == 0), stop=(kt == KTC - 1))
                                next(feeder, None)
                            # stash to SBUF, free psum
                            ao = aop.tile([hd + 1, 2, QT], F32, tag="ao", name="ao")
                            nc.scalar.copy(ao[:, 0, :], ps_o0[0:hd + 1, :])
                            nc.scalar.copy(ao[:, 1, :], ps_o1[0:hd + 1, :])
                            # softmax denominators -> 1/den, broadcast via DRAM
                            dg = dgp.tile([2, QT], F32, tag="dg", name="dg")
                            nc.sync.dma_start(dg[0:1, :], ao[hd:hd + 1, 0, :])
                            nc.sync.dma_start(dg[1:2, :], ao[hd:hd + 1, 1, :])
                            rcp = dgp.tile([2, QT], F32, tag="rcp", name="rcp")
                            nc.vector.reciprocal_approx_fast(rcp[:], dg[:])
                            rd = drowp.tile([2, QT], F32, tag="drow", name="rd")
                            nc.sync.dma_start(rd[:], rcp[:])
                            rep = repp.tile([hd, 2, QT], F32, tag="rep", name="rep")
                            nc.sync.dma_start(rep[:, 0, :],
                                              rd[0:1, :].to_broadcast((hd, QT)))
                            nc.sync.dma_start(rep[:, 1, :],
                                              rd[1:2, :].to_broadcast((hd, QT)))
                            aon = aonp.tile([hd, 2, QT], BF16, tag="aon", name="aon")
                            nc.vector.tensor_tensor(aon[:], ao[0:hd, :, :], rep[:], MULT)
                            # scatter to cc_in[bi]: dst core j = qt*4 + w
                            for w in range(QT // RT):
                                j = qt * (QT // RT) + w
                                nc.sync.dma_start(
                                    ccin_v[bi][j, pp],
                                    aon[:, :, w * RT:(w + 1) * RT])

                def a2a(r):
                    nc.gpsimd.collective_compute(
                        "AllToAll", mybir.AluOpType.bypass,
                        replica_groups=[list(range(n_cores))],
                        ins=[cc_in[r].opt()], outs=[cc_out[r].opt()])

                def empty_gen():
                    return iter(())

                feeder = empty_gen()
                for bi in range(b):
                    qk_t = qkp.tile([P, NQK, s], BF16, tag="qk", name="qk")
                    v_t = vp.tile([P, KTC, EVA], BF16, tag="v", name="v")
                    proj(bi, qk_t, v_t, hid_next)
                    if bi + 1 < b:  # prefetch next batch's activations
                        hid_next = load_hid(bi + 1)
                    sdpa(bi, qk_t, v_t, feeder)
                    for _ in feeder:  # drain leftover o_proj steps
                        pass
                    a2a(bi)
                    feeder = oproj_gen(bi)
                for _ in feeder:  # final round o_proj
                    pass
    nc.finalize()
    return nc


def prep_inputs(cos, sin, hidden_states, w_qkv, w_o,
                b=B, s=S, d=D, h_loc=H_LOC, hd=HD, n_cores=N_CORES):
    """Host-side sharding/layout: returns per-core input maps."""
    BF = ml_dtypes.bfloat16
    cos = np.asarray(cos, dtype=np.float32)
    sin = np.asarray(sin, dtype=np.float32)
    hidden_states = np.asarray(hidden_states, dtype=np.float32)
    w_qkv = np.asarray(w_qkv, dtype=np.float32)
    w_o = np.asarray(w_o, dtype=np.float32)

    T = b * s
    QBLK = h_loc * hd
    HHD = n_cores * QBLK  # total H*HD

    hidden_t = np.ascontiguousarray(hidden_states.reshape(T, d).T).astype(BF)
    w_o_t = np.ascontiguousarray(w_o.T).astype(BF)

    cos_t = cos.T  # [hd, s]
    sin_t = sin.T
    cos2 = np.ascontiguousarray(np.tile(cos_t, (128 // hd, 1)))
    srt = sin_t.copy()
    srt[0:hd // 2] = -sin_t[0:hd // 2]
    sinrot2 = np.ascontiguousarray(np.tile(srt, (128 // hd, 1)))

    maps = []
    for c in range(n_cores):
        wq = w_qkv[c * QBLK:(c + 1) * QBLK] * 0.125
        wk = w_qkv[HHD + c * QBLK:HHD + (c + 1) * QBLK]
        wv = w_qkv[2 * HHD + c * QBLK:2 * HHD + (c + 1) * QBLK]
        w_qk_t = np.ascontiguousarray(np.concatenate([wq, wk], axis=0).T).astype(BF)
        w_v_t = np.ascontiguousarray(wv.T).astype(BF)
        maps.append({
            "hidden_t": hidden_t,
            "w_qk_t": w_qk_t,
            "w_v_t": w_v_t,
            "w_o_t": w_o_t,
            "cos2": cos2,
            "sinrot2": sinrot2,
        })
    return maps


_NC_CACHE = {}


def run(inputs, trace=False, dims=None):
    """Run the distributed kernel. Returns (full_output, BassKernelResults)."""
    dims = dims or dict(b=B, s=S, d=D, h_loc=H_LOC, hd=HD, n_cores=N_CORES)
    key = tuple(sorted(dims.items()))
    if key not in _NC_CACHE:
        _NC_CACHE[key] = build_attention(**dims)
    nc = _NC_CACHE[key]
    maps = prep_inputs(inputs["cos"], inputs["sin"], inputs["hidden_states"],
                       inputs["w_qkv"], inputs["w_o"], **dims)
    res = run_bass_kernel_spmd(nc, maps, list(range(dims["n_cores"])), trace=trace)
    n_cores = dims["n_cores"]
    s = dims["s"]
    RT = s // n_cores
    T = dims["b"] * s
    out = np.empty((T, dims["d"]), dtype=np.float32)
    for c in range(n_cores):
        sl = res.results[c]["out_sl"]
        for r in range(dims["b"]):
            out[r * s + c * RT: r * s + (c + 1) * RT] = sl[r * RT:(r + 1) * RT]
    out = out.reshape(dims["b"], s, dims["d"])
    return out, res


def kernel(**inputs) -> np.ndarray:
    out, _ = run(inputs)
    return out


# revision 10
# speedup vs baseline: 1.2747x; 1.0530x over previous
"""Trainium2 Bass kernel for fused attention (QKV proj + RoPE + SDPA + o_proj).

Sharding: Megatron-style tensor parallel over heads (4 heads/core x 8 cores)
for QKV+SDPA, then per-(batch, query-half) AllToAll rounds switch to token
parallelism for o_proj, so each core emits a disjoint slice of the output.

v3 design:
  - all matmuls bf16 (psum accumulate f32); w_o resident in SBUF
  - ap=512 moving rows everywhere; RoPE batched per [128,512] tile
  - 8 small AllToAll rounds (one per batch x query-half), fired as soon as
    their tokens are normalized; warmup collective absorbs stream setup
  - o_proj matmuls interleaved into the SDPA loop (SDPA is Act-limited)
  - engine balance: Act does only exp; Vector does rope math + psum
    drains; Sync does all DMA issue; GpSimd only triggers collectives
"""
import sys

import numpy as np

try:
    import concourse.bass as bass
except ImportError:  # fresh grading env: make the toolchain importable
    for p in (
        "/root/.axon_site",
        "/root/.axon_site/_ro/trn_rl_repo",
        "/root/.axon_site/_ro/pypackages",
        "/opt/trn_rl_repo",
        "/opt/pypackages",
    ):
        if p not in sys.path:
            sys.path.append(p)
    import concourse.bass as bass

import concourse.bacc as bacc
import concourse.mybir as mybir
import concourse.tile as tile
from concourse.bass_utils import run_bass_kernel_spmd

import ml_dtypes

F32 = mybir.dt.float32
F32R = mybir.dt.float32r
BF16 = mybir.dt.bfloat16
MULT = mybir.AluOpType.mult
ADD = mybir.AluOpType.add
EXP = mybir.ActivationFunctionType.Exp
IDENT = mybir.ActivationFunctionType.Identity

# problem dims (hardcoded for nn_Attention_42846593744909)
B, S, D = 4, 1024, 2048
H, HD = 32, 64
N_CORES = 8
H_LOC = H // N_CORES  # heads per core


def build_attention(b=B, s=S, d=D, h_loc=H_LOC, hd=HD, n_cores=N_CORES):
    """Build the per-core SPMD Bass program. Returns finalized nc."""
    P = 128
    T = b * s                  # total tokens
    DCH = d // P               # contraction chunks for D (16)
    QBLK = h_loc * hd          # 256
    NQK = 2 * QBLK // P        # q+k e-chunks (4)
    EVA = h_loc * (hd + 1)     # v + ones columns (260)
    TH = 512                   # proj token half-batch
    NTH = s // TH              # 2
    QT = 512                   # query tile in SDPA
    NQT = s // QT              # 2
    KTC = s // P               # key chunks of 128 (8)
    ECH = n_cores * QBLK // P  # o_proj contraction chunks (16)
    RT = QT // n_cores         # tokens per core per (batch, qt) round (64)
    ODC = 512                  # o_proj dout chunk (psum bank)
    NDC = d // ODC             # 4
    TS = b * NQT * RT          # output tokens per core (512)

    nc = bacc.Bacc()
    hidden_t = nc.dram_tensor("hidden_t", [d, T], BF16, kind="ExternalInput")
    w_qk_t = nc.dram_tensor("w_qk_t", [d, 2 * QBLK], BF16, kind="ExternalInput")
    w_v_t = nc.dram_tensor("w_v_t", [d, QBLK], BF16, kind="ExternalInput")
    w_o_t = nc.dram_tensor("w_o_t", [n_cores * QBLK, d], BF16, kind="ExternalInput")
    cos2 = nc.dram_tensor("cos2", [P, s], F32, kind="ExternalInput")
    sinrot2 = nc.dram_tensor("sinrot2", [P, s], F32, kind="ExternalInput")
    out_sl = nc.dram_tensor("out_sl", [TS, d], F32, kind="ExternalOutput")

    hid_v = hidden_t[:].rearrange("(c p) t -> p c t", p=P)
    wqk_v = w_qk_t[:].rearrange("(c p) e -> p c e", p=P)
    wv_v = w_v_t[:].rearrange("(c p) e -> p c e", p=P)
    wo_v = w_o_t[:].rearrange("(c p) e -> p c e", p=P)

    with tile.TileContext(nc) as tc:
        with tc.tile_pool(name="dramp", bufs=1, space="DRAM") as dramp:
            ccw_in = dramp.tile([n_cores, 1, 64], BF16, name="ccw_in")
            ccw_out = dramp.tile([n_cores, 1, 64], BF16, name="ccw_out")
            cc_in = [[dramp.tile([n_cores, QBLK, RT], BF16, name=f"cc_in_{r}_{q}")
                      for q in range(NQT)] for r in range(b)]
            cc_out = [[dramp.tile([n_cores, QBLK, RT], BF16, name=f"cc_out_{r}_{q}")
                       for q in range(NQT)] for r in range(b)]
            # scatter view: [dst core j, pp, v, h, t]
            ccin_v = [[t_[:].rearrange("j (pp h v) t -> j pp v h t", pp=2, h=2, v=hd)
                       for t_ in row] for row in cc_in]
            ccout_v = [[t_[:].rearrange("j (ci p) t -> p (j ci) t", p=P)
                        for t_ in row] for row in cc_out]

            with (
                tc.tile_pool(name="tabs", bufs=1) as tabs,
                tc.tile_pool(name="hidp", bufs=2) as hidp,
                tc.tile_pool(name="qkp", bufs=2) as qkp,
                tc.tile_pool(name="vp", bufs=2) as vp,
                tc.tile_pool(name="ropep", bufs=2) as ropep,
                tc.tile_pool(name="expp", bufs=2) as expp,
                tc.tile_pool(name="aop", bufs=2) as aop,
                tc.tile_pool(name="dgp", bufs=1) as dgp,
                tc.tile_pool(name="repp", bufs=1) as repp,
                tc.tile_pool(name="aonp", bufs=1) as aonp,
                tc.tile_pool(name="aslp", bufs=2) as aslp,
                tc.tile_pool(name="obp", bufs=1) as obp,
                tc.tile_pool(name="drowp", bufs=4, space="DRAM") as drowp,
                tc.tile_pool(name="psP", bufs=2, space="PSUM") as psP,
                tc.tile_pool(name="psS", bufs=2, space="PSUM") as psS,
                tc.tile_pool(name="psO", bufs=2, space="PSUM") as psO,
                tc.tile_pool(name="psJ", bufs=2, space="PSUM") as psJ,
            ):
                # ---- static tables / weights (issue order = priority) ----
                wqk_sb = tabs.tile([P, DCH, 2 * QBLK], BF16)
                for ec in range(NQK):
                    nc.sync.dma_start(wqk_sb[:, :, ec * P:(ec + 1) * P],
                                      wqk_v[:, :, ec * P:(ec + 1) * P])

                def load_hid_half(bi, th):
                    t0 = bi * s + th * TH
                    hid_sb = hidp.tile([P, DCH, TH], BF16, tag="hid", name="hid")
                    nc.sync.dma_start(hid_sb[:], hid_v[:, :, t0:t0 + TH])
                    return hid_sb

                hid_next = [load_hid_half(0, 0), load_hid_half(0, 1)]
                # warmup collective: absorb stream setup off the critical path
                nc.gpsimd.collective_compute(
                    "AllToAll", mybir.AluOpType.bypass,
                    replica_groups=[list(range(n_cores))],
                    ins=[ccw_in.opt()], outs=[ccw_out.opt()])
                cos_sb = tabs.tile([P, s], F32)
                sin_sb = tabs.tile([P, s], F32)
                nc.sync.dma_start(cos_sb[:], cos2[:])
                nc.sync.dma_start(sin_sb[:], sinrot2[:])
                wv_sb = tabs.tile([P, DCH, QBLK], BF16)
                nc.sync.dma_start(wv_sb[:], wv_v[:])
                wo_sb = tabs.tile([P, ECH, d], BF16)
                for dc in range(8):
                    nc.sync.dma_start(wo_sb[:, :, dc * 256:(dc + 1) * 256],
                                      wo_v[:, :, dc * 256:(dc + 1) * 256])

                def rope(ps, soff, qk_t, ec):
                    """RoPE a [128, TH] psum tile into qk_t[:, ec, soff:soff+TH]."""
                    raw = ropep.tile([P, TH], F32, tag="raw", name="raw")
                    nc.vector.tensor_copy(raw[:], ps[:])
                    cp = ropep.tile([P, TH], F32, tag="cp", name="cp")
                    nc.vector.tensor_tensor(cp[:], raw[:], cos_sb[:, soff:soff + TH], MULT)
                    sw = ropep.tile([P, TH], F32, tag="sw", name="sw")
                    # rotate_half: swap 32-partition blocks within each head
                    nc.sync.dma_start(sw[0:32, :], raw[32:64, :])
                    nc.sync.dma_start(sw[32:64, :], raw[0:32, :])
                    nc.sync.dma_start(sw[64:96, :], raw[96:128, :])
                    nc.sync.dma_start(sw[96:128, :], raw[64:96, :])
                    nc.vector.tensor_tensor(sw[:], sw[:], sin_sb[:, soff:soff + TH], MULT)
                    nc.vector.tensor_tensor(qk_t[:, ec, soff:soff + TH], cp[:], sw[:], ADD)

                def proj(bi, qk_t, v_t, hid_tiles):
                    """QKV projection + RoPE for batch bi; prefetches bi+1."""
                    for h in range(h_loc):
                        nc.scalar.activation(
                            v_t[:, :, h * (hd + 1) + hd:h * (hd + 1) + hd + 1],
                            wv_sb[:, 0:KTC, 0:1], IDENT, bias=1.0, scale=0.0)
                    nxt = [None, None]
                    for th in range(NTH):
                        s0 = th * TH
                        hid_sb = hid_tiles[th]
                        for ec in range(NQK):
                            ps = psP.tile([P, ODC], F32, tag="psP", name="psqk")
                            for dd in range(DCH):
                                nc.tensor.matmul(
                                    ps[:], lhsT=wqk_sb[:, dd, ec * P:(ec + 1) * P],
                                    rhs=hid_sb[:, dd, :],
                                    start=(dd == 0), stop=(dd == DCH - 1))
                            rope(ps, s0, qk_t, ec)
                        for tsub in range(TH // P):
                            kc = th * (TH // P) + tsub
                            psv = psP.tile([P, ODC], F32, tag="psP", name="psv")
                            for dd in range(DCH):
                                nc.tensor.matmul(
                                    psv[:, 0:QBLK],
                                    lhsT=hid_sb[:, dd, tsub * P:(tsub + 1) * P],
                                    rhs=wv_sb[:, dd, :],
                                    start=(dd == 0), stop=(dd == DCH - 1))
                            for h in range(h_loc):
                                nc.vector.tensor_copy(
                                    v_t[:, kc, h * (hd + 1):h * (hd + 1) + hd],
                                    psv[:, h * hd:(h + 1) * hd])
                        # prefetch next batch's same-half activations
                        if bi + 1 < b:
                            nxt[th] = load_hid_half(bi + 1, th)
                    return nxt

                def oproj_gen(r):
                    """Generator emitting o_proj for round r in 19 steps."""
                    asl = aslp.tile([P, ECH, P], BF16, tag="asl", name=f"asl{r}")
                    for q in range(NQT):
                        nc.sync.dma_start(asl[:, :, q * RT:(q + 1) * RT],
                                          ccout_v[r][q])
                    yield
                    for dcg in range(NDC // 2):
                        pj0 = psJ.tile([P, ODC], F32, tag="psJ", name="pj0")
                        pj1 = psJ.tile([P, ODC], F32, tag="psJ", name="pj1")
                        d0 = (2 * dcg) * ODC
                        d1 = (2 * dcg + 1) * ODC
                        for e2 in range(ECH // 2):
                            for e in (2 * e2, 2 * e2 + 1):
                                nc.tensor.matmul(pj0[:], lhsT=asl[:, e, :],
                                                 rhs=wo_sb[:, e, d0:d0 + ODC],
                                                 start=(e == 0), stop=(e == ECH - 1))
                                nc.tensor.matmul(pj1[:], lhsT=asl[:, e, :],
                                                 rhs=wo_sb[:, e, d1:d1 + ODC],
                                                 start=(e == 0), stop=(e == ECH - 1))
                            yield
                        ob = obp.tile([P, 2 * ODC], F32, tag="ob", name="ob")
                        nc.vector.tensor_copy(ob[:, 0:ODC], pj0[:])
                        nc.vector.tensor_copy(ob[:, ODC:2 * ODC], pj1[:])
                        nc.sync.dma_start(
                            out_sl[r * P:(r + 1) * P, d0:d0 + 2 * ODC], ob[:])
                        yield

                def a2a(r, q):
                    nc.gpsimd.collective_compute(
                        "AllToAll", mybir.AluOpType.bypass,
                        replica_groups=[list(range(n_cores))],
                        ins=[cc_in[r][q].opt()], outs=[cc_out[r][q].opt()])

                def sdpa(bi, qk_t, v_t, feeder):
                    """SDPA for batch bi with o_proj steps interleaved."""
                    slot = 0
                    for qt in range(NQT):
                        q0 = qt * QT
                        for pp in range(h_loc // 2):
                            ps_o0 = psO.tile([P, QT], F32, tag="psO", name="pso0")
                            ps_o1 = psO.tile([P, QT], F32, tag="psO", name="pso1")
                            for kt in range(KTC):
                                ps_s0 = psS.tile([P, QT], F32, tag="psS", name="pss0")
                                ps_s1 = psS.tile([P, QT], F32, tag="psS", name="pss1")
                                nc.tensor.matmul(
                                    ps_s0[:],
                                    lhsT=qk_t[0:64, 2 + pp, kt * P:(kt + 1) * P],
                                    rhs=qk_t[0:64, pp, q0:q0 + QT],
                                    start=True, stop=True)
                                nc.tensor.matmul(
                                    ps_s1[:],
                                    lhsT=qk_t[64:128, 2 + pp, kt * P:(kt + 1) * P],
                                    rhs=qk_t[64:128, pp, q0:q0 + QT],
                                    start=True, stop=True, tile_position=(64, 0))
                                e0 = expp.tile([P, QT], F32R, tag="exp", name="e0")
                                e1 = expp.tile([P, QT], F32R, tag="exp", name="e1")
                                nc.scalar.activation(e0[:], ps_s0[:], EXP)
                                nc.scalar.activation(e1[:], ps_s1[:], EXP)
                                h0 = 2 * pp
                                h1 = 2 * pp + 1
                                nc.tensor.matmul(
                                    ps_o0[0:hd + 1, :],
                                    lhsT=v_t[:, kt, h0 * (hd + 1):(h0 + 1) * (hd + 1)],
                                    rhs=e0[:],
                                    start=(kt == 0), stop=(kt == KTC - 1))
                                nc.tensor.matmul(
                                    ps_o1[0:hd + 1, :],
                                    lhsT=v_t[:, kt, h1 * (hd + 1):(h1 + 1) * (hd + 1)],
                                    rhs=e1[:],
                                    start=(kt == 0), stop=(kt == KTC - 1))
                                slot += 1
                                if slot > 8:  # let round bi-1's A2A land first
                                    next(feeder, None)
                            # stash to SBUF, free psum
                            ao = aop.tile([hd + 1, 2, QT], F32, tag="ao", name="ao")
                            nc.vector.tensor_copy(ao[:, 0, :], ps_o0[0:hd + 1, :])
                            nc.vector.tensor_copy(ao[:, 1, :], ps_o1[0:hd + 1, :])
                            # softmax denominators -> 1/den, broadcast via DRAM
                            dg = dgp.tile([2, QT], F32, tag="dg", name="dg")
                            nc.sync.dma_start(dg[0:1, :], ao[hd:hd + 1, 0, :])
                            nc.sync.dma_start(dg[1:2, :], ao[hd:hd + 1, 1, :])
                            rcp = dgp.tile([2, QT], F32, tag="rcp", name="rcp")
                            nc.vector.reciprocal_approx_fast(rcp[:], dg[:])
                            rd = drowp.tile([2, QT], F32, tag="drow", name="rd")
                            nc.sync.dma_start(rd[:], rcp[:])
                            rep = repp.tile([hd, 2, QT], F32, tag="rep", name="rep")
                            nc.sync.dma_start(rep[:, 0, :],
                                              rd[0:1, :].to_broadcast((hd, QT)))
                            nc.sync.dma_start(rep[:, 1, :],
                                              rd[1:2, :].to_broadcast((hd, QT)))
                            aon = aonp.tile([hd, 2, QT], BF16, tag="aon", name="aon")
                            nc.vector.tensor_tensor(aon[:], ao[0:hd, :, :], rep[:], MULT)
                            # scatter to cc_in[bi][qt]: dst core j gets 64 toks
                            for j in range(n_cores):
                                nc.sync.dma_start(
                                    ccin_v[bi][qt][j, pp],
                                    aon[:, :, j * RT:(j + 1) * RT])
                        a2a(bi, qt)

                def empty_gen():
                    return iter(())

                feeder = empty_gen()
                for bi in range(b):
                    qk_t = qkp.tile([P, NQK, s], BF16, tag="qk", name="qk")
                    v_t = vp.tile([P, KTC, EVA], F32R, tag="v", name="v")
                    hid_next = proj(bi, qk_t, v_t, hid_next)
                    sdpa(bi, qk_t, v_t, feeder)
                    for _ in feeder:  # drain leftover o_proj steps
                        pass
                    feeder = oproj_gen(bi)
                for _ in feeder:  # final round o_proj
                    pass
    nc.finalize()
    return nc


def prep_inputs(cos, sin, hidden_states, w_qkv, w_o,
                b=B, s=S, d=D, h_loc=H_LOC, hd=HD, n_cores=N_CORES):
    """Host-side sharding/layout: returns per-core input maps."""
    BF = ml_dtypes.bfloat16
    cos = np.asarray(cos, dtype=np.float32)
    sin = np.asarray(sin, dtype=np.float32)
    hidden_states = np.asarray(hidden_states, dtype=np.float32)
    w_qkv = np.asarray(w_qkv, dtype=np.float32)
    w_o = np.asarray(w_o, dtype=np.float32)

    T = b * s
    QBLK = h_loc * hd
    HHD = n_cores * QBLK  # total H*HD

    hidden_t = np.ascontiguousarray(hidden_states.reshape(T, d).T).astype(BF)
    w_o_t = np.ascontiguousarray(w_o.T).astype(BF)

    cos_t = cos.T  # [hd, s]
    sin_t = sin.T
    cos2 = np.ascontiguousarray(np.tile(cos_t, (128 // hd, 1)))
    srt = sin_t.copy()
    srt[0:hd // 2] = -sin_t[0:hd // 2]
    sinrot2 = np.ascontiguousarray(np.tile(srt, (128 // hd, 1)))

    maps = []
    for c in range(n_cores):
        wq = w_qkv[c * QBLK:(c + 1) * QBLK] * 0.125
        wk = w_qkv[HHD + c * QBLK:HHD + (c + 1) * QBLK]
        wv = w_qkv[2 * HHD + c * QBLK:2 * HHD + (c + 1) * QBLK]
        w_qk_t = np.ascontiguousarray(np.concatenate([wq, wk], axis=0).T).astype(BF)
        w_v_t = np.ascontiguousarray(wv.T).astype(BF)
        maps.append({
            "hidden_t": hidden_t,
            "w_qk_t": w_qk_t,
            "w_v_t": w_v_t,
            "w_o_t": w_o_t,
            "cos2": cos2,
            "sinrot2": sinrot2,
        })
    return maps


_NC_CACHE = {}


def run(inputs, trace=False, dims=None):
    """Run the distributed kernel. Returns (full_output, BassKernelResults)."""
    dims = dims or dict(b=B, s=S, d=D, h_loc=H_LOC, hd=HD, n_cores=N_CORES)
    key = tuple(sorted(dims.items()))
    if key not in _NC_CACHE:
        _NC_CACHE[key] = build_attention(**dims)
    nc = _NC_CACHE[key]
    maps = prep_inputs(inputs["cos"], inputs["sin"], inputs["hidden_states"],
                       inputs["w_qkv"], inputs["w_o"], **dims)
    res = run_bass_kernel_spmd(nc, maps, list(range(dims["n_cores"])), trace=trace)
    n_cores = dims["n_cores"]
    s = dims["s"]
    QT = 512
    RT = QT // n_cores  # 64
    T = dims["b"] * s
    out = np.empty((T, dims["d"]), dtype=np.float32)
    for c in range(n_cores):
        sl = res.results[c]["out_sl"]
        for r in range(dims["b"]):
            for q in range(s // QT):
                g0 = r * s + q * QT + c * RT
                o0 = r * 128 + q * RT
                out[g0:g0 + RT] = sl[o0:o0 + RT]
    out = out.reshape(dims["b"], s, dims["d"])
    return out, res


def kernel(**inputs) -> np.ndarray:
    out, _ = run(inputs)
    return out


# revision 12
# speedup vs baseline: 1.2935x; 1.0147x over previous
"""Trainium2 Bass kernel for fused attention (QKV proj + RoPE + SDPA + o_proj).

Sharding: Megatron-style tensor parallel over heads (4 heads/core x 8 cores)
for QKV+SDPA, then per-(batch, query-half) AllToAll rounds switch to token
parallelism for o_proj, so each core emits a disjoint slice of the output.

v3 design:
  - all matmuls bf16 (psum accumulate f32); w_o resident in SBUF
  - ap=512 moving rows everywhere; RoPE batched per [128,512] tile
  - 8 small AllToAll rounds (one per batch x query-half), fired as soon as
    their tokens are normalized; warmup collective absorbs stream setup
  - o_proj matmuls interleaved into the SDPA loop (SDPA is Act-limited)
  - engine balance: Act does only exp; Vector does rope math + psum
    drains; Sync does all DMA issue; GpSimd only triggers collectives
"""
import sys

import numpy as np

try:
    import concourse.bass as bass
except ImportError:  # fresh grading env: make the toolchain importable
    for p in (
        "/root/.axon_site",
        "/root/.axon_site/_ro/trn_rl_repo",
        "/root/.axon_site/_ro/pypackages",
        "/opt/trn_rl_repo",
        "/opt/pypackages",
    ):
        if p not in sys.path:
            sys.path.append(p)
    import concourse.bass as bass

import concourse.bacc as bacc
import concourse.mybir as mybir
import concourse.tile as tile
from concourse.bass_utils import run_bass_kernel_spmd

import ml_dtypes

F32 = mybir.dt.float32
F32R = mybir.dt.float32r
BF16 = mybir.dt.bfloat16
MULT = mybir.AluOpType.mult
ADD = mybir.AluOpType.add
EXP = mybir.ActivationFunctionType.Exp
IDENT = mybir.ActivationFunctionType.Identity

# problem dims (hardcoded for nn_Attention_42846593744909)
B, S, D = 4, 1024, 2048
H, HD = 32, 64
N_CORES = 8
H_LOC = H // N_CORES  # heads per core


def build_attention(b=B, s=S, d=D, h_loc=H_LOC, hd=HD, n_cores=N_CORES):
    """Build the per-core SPMD Bass program. Returns finalized nc."""
    P = 128
    T = b * s                  # total tokens
    DCH = d // P               # contraction chunks for D (16)
    QBLK = h_loc * hd          # 256
    NQK = 2 * QBLK // P        # q+k e-chunks (4)
    EVA = h_loc * (hd + 1)     # v + ones columns (260)
    TH = 512                   # proj token half-batch
    NTH = s // TH              # 2
    QT = 512                   # query tile in SDPA
    NQT = s // QT              # 2
    KTC = s // P               # key chunks of 128 (8)
    ECH = n_cores * QBLK // P  # o_proj contraction chunks (16)
    RT = QT // n_cores         # tokens per core per (batch, qt) round (64)
    ODC = 512                  # o_proj dout chunk (psum bank)
    NDC = d // ODC             # 4
    TS = b * NQT * RT          # output tokens per core (512)

    nc = bacc.Bacc()
    hidden_t = nc.dram_tensor("hidden_t", [d, T], BF16, kind="ExternalInput")
    w_qk_t = nc.dram_tensor("w_qk_t", [d, 2 * QBLK], BF16, kind="ExternalInput")
    w_v_t = nc.dram_tensor("w_v_t", [d, QBLK], BF16, kind="ExternalInput")
    w_o_t = nc.dram_tensor("w_o_t", [n_cores * QBLK, d], BF16, kind="ExternalInput")
    cos2 = nc.dram_tensor("cos2", [P, s], F32, kind="ExternalInput")
    sinrot2 = nc.dram_tensor("sinrot2", [P, s], F32, kind="ExternalInput")
    out_sl = nc.dram_tensor("out_sl", [TS, d], F32, kind="ExternalOutput")

    hid_v = hidden_t[:].rearrange("(c p) t -> p c t", p=P)
    wqk_v = w_qk_t[:].rearrange("(c p) e -> p c e", p=P)
    wv_v = w_v_t[:].rearrange("(c p) e -> p c e", p=P)
    wo_v = w_o_t[:].rearrange("(c p) e -> p c e", p=P)

    with tile.TileContext(nc) as tc:
        with tc.tile_pool(name="dramp", bufs=1, space="DRAM") as dramp:
            cc_in = [[dramp.tile([n_cores, QBLK, RT], BF16, name=f"cc_in_{r}_{q}")
                      for q in range(NQT)] for r in range(b)]
            cc_out = [[dramp.tile([n_cores, QBLK, RT], BF16, name=f"cc_out_{r}_{q}")
                       for q in range(NQT)] for r in range(b)]
            # scatter view: [pp, h, v, dst core j, t]
            ccin_v = [[t_[:].rearrange("j (pp h v) t -> pp h v j t", pp=2, h=2, v=hd)
                       for t_ in row] for row in cc_in]
            ccout_v = [[t_[:].rearrange("j (ci p) t -> p (j ci) t", p=P)
                        for t_ in row] for row in cc_out]

            with (
                tc.tile_pool(name="tabs", bufs=1) as tabs,
                tc.tile_pool(name="hidp", bufs=2) as hidp,
                tc.tile_pool(name="qkp", bufs=2) as qkp,
                tc.tile_pool(name="vp", bufs=2) as vp,
                tc.tile_pool(name="ropep", bufs=2) as ropep,
                tc.tile_pool(name="expp", bufs=2) as expp,
                tc.tile_pool(name="aop", bufs=2) as aop,
                tc.tile_pool(name="dgp", bufs=1) as dgp,
                tc.tile_pool(name="repp", bufs=1) as repp,
                tc.tile_pool(name="aonp", bufs=1) as aonp,
                tc.tile_pool(name="aslp", bufs=2) as aslp,
                tc.tile_pool(name="obp", bufs=1) as obp,
                tc.tile_pool(name="drowp", bufs=4, space="DRAM") as drowp,
                tc.tile_pool(name="psP", bufs=2, space="PSUM") as psP,
                tc.tile_pool(name="psS", bufs=4, space="PSUM") as psS,
                tc.tile_pool(name="psO", bufs=2, space="PSUM") as psO,
            ):
                # ---- static tables / weights (issue order = priority) ----
                wqk_sb = tabs.tile([P, DCH, 2 * QBLK], BF16)
                for ec in range(NQK):
                    nc.sync.dma_start(wqk_sb[:, :, ec * P:(ec + 1) * P],
                                      wqk_v[:, :, ec * P:(ec + 1) * P])

                def load_hid_half(bi, th):
                    t0 = bi * s + th * TH
                    hid_sb = hidp.tile([P, DCH, TH], BF16, tag="hid", name="hid")
                    nc.sync.dma_start(hid_sb[:], hid_v[:, :, t0:t0 + TH])
                    return hid_sb

                hid_next = [load_hid_half(0, 0), load_hid_half(0, 1)]
                cos_sb = tabs.tile([P, s], F32)
                sin_sb = tabs.tile([P, s], F32)
                nc.sync.dma_start(cos_sb[:], cos2[:])
                nc.sync.dma_start(sin_sb[:], sinrot2[:])
                wv_sb = tabs.tile([P, DCH, QBLK], BF16)
                nc.sync.dma_start(wv_sb[:], wv_v[:])
                wo_sb = tabs.tile([P, ECH, d], BF16)
                for dc in range(8):
                    nc.sync.dma_start(wo_sb[:, :, dc * 256:(dc + 1) * 256],
                                      wo_v[:, :, dc * 256:(dc + 1) * 256])

                def rope(ps, soff, qk_t, ec):
                    """RoPE a [128, TH] psum tile into qk_t[:, ec, soff:soff+TH]."""
                    raw = ropep.tile([P, TH], F32, tag="raw", name="raw")
                    nc.vector.tensor_copy(raw[:], ps[:])
                    cp = ropep.tile([P, TH], F32, tag="cp", name="cp")
                    nc.vector.tensor_tensor(cp[:], raw[:], cos_sb[:, soff:soff + TH], MULT)
                    sw = ropep.tile([P, TH], F32, tag="sw", name="sw")
                    # rotate_half: swap 32-partition blocks within each head
                    nc.sync.dma_start(sw[0:32, :], raw[32:64, :])
                    nc.sync.dma_start(sw[32:64, :], raw[0:32, :])
                    nc.sync.dma_start(sw[64:96, :], raw[96:128, :])
                    nc.sync.dma_start(sw[96:128, :], raw[64:96, :])
                    nc.vector.tensor_tensor(sw[:], sw[:], sin_sb[:, soff:soff + TH], MULT)
                    nc.vector.tensor_tensor(qk_t[:, ec, soff:soff + TH], cp[:], sw[:], ADD)

                def proj(bi, qk_t, v_t, hid_tiles):
                    """QKV projection + RoPE for batch bi; prefetches bi+1."""
                    for h in range(h_loc):
                        nc.scalar.activation(
                            v_t[:, :, h * (hd + 1) + hd:h * (hd + 1) + hd + 1],
                            wv_sb[:, 0:KTC, 0:1], IDENT, bias=1.0, scale=0.0)
                    nxt = [None, None]
                    for th in range(NTH):
                        s0 = th * TH
                        hid_sb = hid_tiles[th]
                        for ec in range(NQK):
                            ps = psP.tile([P, ODC], F32, tag="psP", name="psqk")
                            for dd in range(DCH):
                                nc.tensor.matmul(
                                    ps[:], lhsT=wqk_sb[:, dd, ec * P:(ec + 1) * P],
                                    rhs=hid_sb[:, dd, :],
                                    start=(dd == 0), stop=(dd == DCH - 1))
                            rope(ps, s0, qk_t, ec)
                        for tsub in range(TH // P):
                            kc = th * (TH // P) + tsub
                            psv = psP.tile([P, ODC], F32, tag="psP", name="psv")
                            for dd in range(DCH):
                                nc.tensor.matmul(
                                    psv[:, 0:QBLK],
                                    lhsT=hid_sb[:, dd, tsub * P:(tsub + 1) * P],
                                    rhs=wv_sb[:, dd, :],
                                    start=(dd == 0), stop=(dd == DCH - 1))
                            for h in range(h_loc):
                                nc.vector.tensor_copy(
                                    v_t[:, kc, h * (hd + 1):h * (hd + 1) + hd],
                                    psv[:, h * hd:(h + 1) * hd])
                        # prefetch next batch's same-half activations
                        if bi + 1 < b:
                            nxt[th] = load_hid_half(bi + 1, th)
                    return nxt

                def oproj_gen(r):
                    """Generator emitting o_proj for round r in 19 steps."""
                    asl = aslp.tile([P, ECH, P], BF16, tag="asl", name=f"asl{r}")
                    for q in range(NQT):
                        nc.gpsimd.dma_start(asl[:, :, q * RT:(q + 1) * RT],
                                            ccout_v[r][q])
                    yield
                    for dcg in range(NDC // 2):
                        pj0 = psP.tile([P, ODC], F32, tag="psP", name="pj0")
                        pj1 = psP.tile([P, ODC], F32, tag="psP", name="pj1")
                        d0 = (2 * dcg) * ODC
                        d1 = (2 * dcg + 1) * ODC
                        for e2 in range(ECH // 2):
                            for e in (2 * e2, 2 * e2 + 1):
                                nc.tensor.matmul(pj0[:], lhsT=asl[:, e, :],
                                                 rhs=wo_sb[:, e, d0:d0 + ODC],
                                                 start=(e == 0), stop=(e == ECH - 1))
                                nc.tensor.matmul(pj1[:], lhsT=asl[:, e, :],
                                                 rhs=wo_sb[:, e, d1:d1 + ODC],
                                                 start=(e == 0), stop=(e == ECH - 1))
                            yield
                        ob = obp.tile([P, 2 * ODC], F32, tag="ob", name="ob")
                        nc.vector.tensor_copy(ob[:, 0:ODC], pj0[:])
                        nc.vector.tensor_copy(ob[:, ODC:2 * ODC], pj1[:])
                        nc.sync.dma_start(
                            out_sl[r * P:(r + 1) * P, d0:d0 + 2 * ODC], ob[:])
                        yield

                def a2a(r, q):
                    nc.gpsimd.collective_compute(
                        "AllToAll", mybir.AluOpType.bypass,
                        replica_groups=[list(range(n_cores))],
                        ins=[cc_in[r][q].opt()], outs=[cc_out[r][q].opt()])

                def sdpa(bi, qk_t, v_t, feeder):
                    """SDPA for batch bi with o_proj steps interleaved."""
                    slot = 0
                    for qt in range(NQT):
                        q0 = qt * QT
                        for pp in range(h_loc // 2):
                            ps_o0 = psO.tile([P, QT], F32, tag="psO", name="pso0")
                            ps_o1 = psO.tile([P, QT], F32, tag="psO", name="pso1")
                            for kt in range(KTC):
                                ps_s0 = psS.tile([P, QT], F32, tag="psS", name="pss0")
                                ps_s1 = psS.tile([P, QT], F32, tag="psS", name="pss1")
                                nc.tensor.matmul(
                                    ps_s0[:],
                                    lhsT=qk_t[0:64, 2 + pp, kt * P:(kt + 1) * P],
                                    rhs=qk_t[0:64, pp, q0:q0 + QT],
                                    start=True, stop=True)
                                nc.tensor.matmul(
                                    ps_s1[:],
                                    lhsT=qk_t[64:128, 2 + pp, kt * P:(kt + 1) * P],
                                    rhs=qk_t[64:128, pp, q0:q0 + QT],
                                    start=True, stop=True, tile_position=(64, 0))
                                e0 = expp.tile([P, QT], F32R, tag="exp", name="e0")
                                e1 = expp.tile([P, QT], F32R, tag="exp", name="e1")
                                nc.scalar.activation(e0[:], ps_s0[:], EXP)
                                nc.scalar.activation(e1[:], ps_s1[:], EXP)
                                h0 = 2 * pp
                                h1 = 2 * pp + 1
                                nc.tensor.matmul(
                                    ps_o0[0:hd + 1, :],
                                    lhsT=v_t[:, kt, h0 * (hd + 1):(h0 + 1) * (hd + 1)],
                                    rhs=e0[:],
                                    start=(kt == 0), stop=(kt == KTC - 1))
                                nc.tensor.matmul(
                                    ps_o1[0:hd + 1, :],
                                    lhsT=v_t[:, kt, h1 * (hd + 1):(h1 + 1) * (hd + 1)],
                                    rhs=e1[:],
                                    start=(kt == 0), stop=(kt == KTC - 1))
                                slot += 1
                                if slot > 8:  # let round bi-1's A2A land first
                                    next(feeder, None)
                            # stash to SBUF, free psum
                            ao = aop.tile([hd + 1, 2, QT], F32, tag="ao", name="ao")
                            nc.vector.tensor_copy(ao[:, 0, :], ps_o0[0:hd + 1, :])
                            nc.vector.tensor_copy(ao[:, 1, :], ps_o1[0:hd + 1, :])
                            # softmax denominators -> 1/den, broadcast via DRAM
                            dg = dgp.tile([2, QT], F32, tag="dg", name="dg")
                            nc.sync.dma_start(dg[:], ao[hd:hd + 1, :, :])
                            rcp = dgp.tile([2, QT], F32, tag="rcp", name="rcp")
                            nc.vector.reciprocal_approx_fast(rcp[:], dg[:])
                            rd = drowp.tile([2, QT], F32, tag="drow", name="rd")
                            nc.sync.dma_start(rd[:], rcp[:])
                            rep = repp.tile([hd, 2, QT], F32, tag="rep", name="rep")
                            nc.sync.dma_start(rep[:, 0, :],
                                              rd[0:1, :].to_broadcast((hd, QT)))
                            nc.sync.dma_start(rep[:, 1, :],
                                              rd[1:2, :].to_broadcast((hd, QT)))
                            aon = aonp.tile([hd, 2, QT], BF16, tag="aon", name="aon")
                            nc.vector.tensor_tensor(aon[:], ao[0:hd, :, :], rep[:], MULT)
                            # scatter to cc_in[bi][qt] (one DMA per head)
                            for h in range(2):
                                nc.sync.dma_start(
                                    ccin_v[bi][qt][pp, h],
                                    aon[:, h, :].rearrange("v (j t) -> v j t",
                                                           j=n_cores))
                        a2a(bi, qt)

                def empty_gen():
                    return iter(())

                feeder = empty_gen()
                for bi in range(b):
                    qk_t = qkp.tile([P, NQK, s], BF16, tag="qk", name="qk")
                    v_t = vp.tile([P, KTC, EVA], F32R, tag="v", name="v")
                    hid_next = proj(bi, qk_t, v_t, hid_next)
                    sdpa(bi, qk_t, v_t, feeder)
                    for _ in feeder:  # drain leftover o_proj steps
                        pass
                    feeder = oproj_gen(bi)
                    next(feeder, None)  # emit asl DMA (gpsimd) eagerly
                for _ in feeder:  # final round o_proj
                    pass
    nc.finalize()
    return nc


def prep_inputs(cos, sin, hidden_states, w_qkv, w_o,
                b=B, s=S, d=D, h_loc=H_LOC, hd=HD, n_cores=N_CORES):
    """Host-side sharding/layout: returns per-core input maps."""
    BF = ml_dtypes.bfloat16
    cos = np.asarray(cos, dtype=np.float32)
    sin = np.asarray(sin, dtype=np.float32)
    hidden_states = np.asarray(hidden_states, dtype=np.float32)
    w_qkv = np.asarray(w_qkv, dtype=np.float32)
    w_o = np.asarray(w_o, dtype=np.float32)

    T = b * s
    QBLK = h_loc * hd
    HHD = n_cores * QBLK  # total H*HD

    hidden_t = np.ascontiguousarray(hidden_states.reshape(T, d).T).astype(BF)
    w_o_t = np.ascontiguousarray(w_o.T).astype(BF)

    cos_t = cos.T  # [hd, s]
    sin_t = sin.T
    cos2 = np.ascontiguousarray(np.tile(cos_t, (128 // hd, 1)))
    srt = sin_t.copy()
    srt[0:hd // 2] = -sin_t[0:hd // 2]
    sinrot2 = np.ascontiguousarray(np.tile(srt, (128 // hd, 1)))

    maps = []
    for c in range(n_cores):
        wq = w_qkv[c * QBLK:(c + 1) * QBLK] * 0.125
        wk = w_qkv[HHD + c * QBLK:HHD + (c + 1) * QBLK]
        wv = w_qkv[2 * HHD + c * QBLK:2 * HHD + (c + 1) * QBLK]
        w_qk_t = np.ascontiguousarray(np.concatenate([wq, wk], axis=0).T).astype(BF)
        w_v_t = np.ascontiguousarray(wv.T).astype(BF)
        maps.append({
            "hidden_t": hidden_t,
            "w_qk_t": w_qk_t,
            "w_v_t": w_v_t,
            "w_o_t": w_o_t,
            "cos2": cos2,
            "sinrot2": sinrot2,
        })
    return maps


_NC_CACHE = {}


def run(inputs, trace=False, dims=None):
    """Run the distributed kernel. Returns (full_output, BassKernelResults)."""
    dims = dims or dict(b=B, s=S, d=D, h_loc=H_LOC, hd=HD, n_cores=N_CORES)
    key = tuple(sorted(dims.items()))
    if key not in _NC_CACHE:
        _NC_CACHE[key] = build_attention(**dims)
    nc = _NC_CACHE[key]
    maps = prep_inputs(inputs["cos"], inputs["sin"], inputs["hidden_states"],
                       inputs["w_qkv"], inputs["w_o"], **dims)
    res = run_bass_kernel_spmd(nc, maps, list(range(dims["n_cores"])), trace=trace)
    n_cores = dims["n_cores"]
    s = dims["s"]
    QT = 512
    RT = QT // n_cores  # 64
    T = dims["b"] * s
    out = np.empty((T, dims["d"]), dtype=np.float32)
    for c in range(n_cores):
        sl = res.results[c]["out_sl"]
        for r in range(dims["b"]):
            for q in range(s // QT):
                g0 = r * s + q * QT + c * RT
                o0 = r * 128 + q * RT
                out[g0:g0 + RT] = sl[o0:o0 + RT]
    out = out.reshape(dims["b"], s, dims["d"])
    return out, res


def kernel(**inputs) -> np.ndarray:
    out, _ = run(inputs)
    return out
